# revision 1
# baseline (speedup 1.0000x reference)
"""Multi-head attention (B=1, L=4096, C=512, H=8, D=64) on 8 TRN2 NeuronCores.

Sharding: head-parallel — core h computes head h end-to-end (QKV projection
for its head, softmax attention, and its partial contribution to the output
projection). Host sums the 8 partial output projections and adds the bias.

Per-core kernel (all operands pre-transposed on host; zero on-device
transposes):
  stage 1 (fused): psum [q;k] = [wq|wk].T @ xT-slices -> qkT tile
           (q rows 0-63, k rows 64-127); small SBUF-SBUF DMAs build the
           crossed copy kqT (k rows 0-63, q rows 64-127) so the score
           matmuls can run PAIRED in disjoint PE row-groups (K=64 each,
           tile_position (0,0) / (64,0) derived from base partitions).
  stage 2: v[L,D] = xT-tiles.T @ wv, augmented with a ones column so the
           PV matmul also accumulates softmax row-sums.
  attention per 512-wide query slice, key-tile pairs (jA, jB):
     stp[:, :512]   = kqT[0:64,jA].T @ qkT[0:64, i]     (PE rows 0-63)
     stp[:, 512:]   = qkT[64:,jB].T @ kqT[64:, i]       (PE rows 64-127)
     e = exp(stp)  [128,1024] PSUM->SBUF bf16           (ScalarE)
     pvp[:, :512]  += v_aug[jA].T @ e[:, :512]           (bank 0)
     pvp[:, 512:]  += v_aug[jB].T @ e[:, 512:]           (bank 1)
  tail per query slice: ao = pvp-halves summed (bf16), denominators
     bounced through DRAM to partition-major layout, reciprocal on VectorE.
  deferred end phase: out-proj matmuls over all 32 l-tiles, per-row scale
     by the softmax reciprocals (per-partition there), DMA out fp32.
"""

import numpy as np
import ml_dtypes

L, C, D, H = 4096, 512, 64, 8
N_CORES = 8
P = 128

_BF16 = ml_dtypes.bfloat16


def build_nc(L=L, C=C, D=D, reps=1, ablate=(), st_bufs=3, pv_bufs=1, e_bufs=6, split_exp=False, ni=1):
    import contextlib
    import concourse.bacc as bacc
    import concourse.mybir as mybir
    import concourse.tile as tile

    f32 = mybir.dt.float32
    bf16 = mybir.dt.bfloat16
    Exp = mybir.ActivationFunctionType.Exp

    CT = C // P          # contraction tiles over channels (4)
    LT = L // P          # key tiles (32)
    NSL = L // 512       # 512-wide l-slices (8)
    NPAIR = LT // 2      # key tile pairs (16)

    nc = bacc.Bacc("TRN2", target_bir_lowering=False, debug=False)

    xt_d = nc.dram_tensor("xt", [C, L], bf16, kind="ExternalInput")
    wqk_d = nc.dram_tensor("wqk", [C, P], bf16, kind="ExternalInput")
    wv_d = nc.dram_tensor("wv", [C, D], bf16, kind="ExternalInput")
    wo_d = nc.dram_tensor("wo", [D, C], bf16, kind="ExternalInput")
    y_d = nc.dram_tensor("y", [L, C], f32, kind="ExternalOutput")

    with tile.TileContext(nc) as tc:
        with (
            tc.tile_pool(name="const", bufs=1) as constp,
            tc.tile_pool(name="xtp", bufs=1) as xtp,
            tc.tile_pool(name="qkv", bufs=1) as qkvp,
            tc.tile_pool(name="exps", bufs=e_bufs) as expp,
            tc.tile_pool(name="aon", bufs=2) as aop,
            tc.tile_pool(name="rowp", bufs=2) as rowp,
            tc.tile_pool(name="yp", bufs=4) as yp,
            tc.tile_pool(name="drs", bufs=2, space="DRAM") as drsp,
            tc.tile_pool(name="st_ps", bufs=st_bufs, space="PSUM") as stps,
            tc.tile_pool(name="pv_ps", bufs=pv_bufs, space="PSUM") as pvps,
        ):
            # ---- load inputs to SBUF
            xt_sb = []
            for ct in range(CT):
                t = xtp.tile([P, L], bf16, name=f"xt{ct}", tag=f"xt{ct}")
                nc.sync.dma_start(t[:], xt_d[ct * P : (ct + 1) * P, :])
                xt_sb.append(t)
            wqk_sb = constp.tile([P, CT, P], bf16, name="wqk_sb", tag="wqk")
            wv_sb = constp.tile([P, CT, D], bf16, name="wv_sb", tag="wv")
            for ct in range(CT):
                nc.sync.dma_start(wqk_sb[:, ct, :], wqk_d[ct * P : (ct + 1) * P, :])
                nc.sync.dma_start(wv_sb[:, ct, :], wv_d[ct * P : (ct + 1) * P, :])
            wo_sb = constp.tile([D, C], bf16, name="wo_sb", tag="wo")
            nc.sync.dma_start(wo_sb[:], wo_d[:])

            # ---- stage 1: qkT = [q;k] and crossed copy kqT = [k;q], [128, L]
            qkT = qkvp.tile([P, L], bf16, name="qkT", tag="qkT")
            kqT = qkvp.tile([P, L], bf16, name="kqT", tag="kqT")
            v_sb = qkvp.tile([P, LT, D + 1], bf16, name="v_sb", tag="v")
            ao_all = qkvp.tile([D, L], bf16, name="ao_all", tag="ao_all")
            rec_all = qkvp.tile([P, LT], f32, name="rec_all", tag="rec_all")
            rep_ctx = tc.For_i(0, reps, 1) if reps > 1 else contextlib.nullcontext()
            with rep_ctx:
              for ls in range(NSL):
                sl = slice(ls * 512, (ls + 1) * 512)
                ps1 = stps.tile([P, 1024], f32, name="ps1", tag="st")
                for ct in range(CT):
                    nc.tensor.matmul(
                        ps1[:, :512],
                        wqk_sb[:, ct, :],
                        xt_sb[ct][:, sl],
                        start=(ct == 0),
                        stop=(ct == CT - 1),
                    )
                nc.vector.tensor_copy(qkT[:, sl], ps1[:, :512])
                # crossed copy via SBUF->SBUF DMA (partition swap)
                nc.sync.dma_start(kqT[:D, sl], qkT[D:, sl])
                nc.sync.dma_start(kqT[D:, sl], qkT[:D, sl])

              # ---- stage 2: v [L, D] bf16 (+ ones column for row-sums)
              for lt in range(LT):
                ps2 = stps.tile([P, 1024], f32, name="ps2", tag="st")
                for ct in range(CT):
                    nc.tensor.matmul(
                        ps2[:, :D],
                        xt_sb[ct][:, lt * P : (lt + 1) * P],
                        wv_sb[:, ct, :],
                        start=(ct == 0),
                        stop=(ct == CT - 1),
                    )
                nc.vector.tensor_copy(v_sb[:, lt, :D], ps2[:, :D])
              nc.vector.memset(v_sb[:, :, D], 1.0)

              # ---- attention, per 512-wide query slice; out-proj deferred
              if True:
                for g in range(NSL // ni):
                  gslices = [g * ni + t for t in range(ni)]
                  pvs = {}
                  for isl in gslices:
                      pvp = pvps.tile([D + 1, 1024], f32, name="pvp", tag="pv")
                      pvs[isl] = pvp
                  for m in range(NPAIR):
                   for isl in gslices:
                    isx = slice(isl * 512, (isl + 1) * 512)
                    pvp = pvs[isl]
                    if True:
                        jA, jB = 2 * m, 2 * m + 1
                        stp = stps.tile([P, 1024], f32, name="stp", tag="st")
                        if "st" not in ablate:
                            nc.tensor.matmul(
                                stp[:, :512],
                                kqT[:D, jA * P : (jA + 1) * P],
                                qkT[:D, isx],
                                start=True,
                                stop=True,
                            )
                            nc.tensor.matmul(
                                stp[:, 512:],
                                qkT[D:, jB * P : (jB + 1) * P],
                                kqT[D:, isx],
                                start=True,
                                stop=True,
                            )
                        e = expp.tile([P, 1024], bf16, name="e", tag="e")
                        if split_exp:
                            for half, j_ in ((0, jA), (1, jB)):
                                hs = slice(half * 512, (half + 1) * 512)
                                if "exp" not in ablate:
                                    nc.scalar.activation(e[:, hs], stp[:, hs], Exp)
                                if "pv" not in ablate:
                                    nc.tensor.matmul(
                                        pvp[:, hs],
                                        v_sb[:, j_, :],
                                        e[:, hs],
                                        start=(m == 0),
                                        stop=(m == NPAIR - 1),
                                    )
                        else:
                            if "exp" not in ablate:
                                nc.scalar.activation(e[:], stp[:], Exp)
                            if "pv" not in ablate:
                                nc.tensor.matmul(
                                    pvp[:, :512],
                                    v_sb[:, jA, :],
                                    e[:, :512],
                                    start=(m == 0),
                                    stop=(m == NPAIR - 1),
                                )
                                nc.tensor.matmul(
                                    pvp[:, 512:],
                                    v_sb[:, jB, :],
                                    e[:, 512:],
                                    start=(m == 0),
                                    stop=(m == NPAIR - 1),
                                )
                  for isl in gslices:
                    isx = slice(isl * 512, (isl + 1) * 512)
                    pvp = pvs[isl]
                    if "tail" in ablate:
                        continue
                    # combine the two pvp bank-halves; row D = denominators
                    aoh = aop.tile([D, 512], f32, name="aoh", tag="aoh")
                    nc.vector.tensor_copy(aoh[:], pvp[:D, :512])
                    nc.vector.tensor_add(ao_all[:, isx], aoh[:], pvp[:D, 512:])
                    r1 = rowp.tile([1, 512], f32, name="r1", tag="r1")
                    nc.vector.tensor_copy(r1[:], pvp[D : D + 1, :512])
                    rec_row = rowp.tile([1, 512], f32, name="rec_row", tag="rr")
                    nc.vector.tensor_add(rec_row[:], r1[:], pvp[D : D + 1, 512:])
                    nc.vector.reciprocal(rec_row[:], rec_row[:])
                    dr = drsp.tile([512], f32, name="dr", tag="dr")
                    nc.sync.dma_start(dr[:], rec_row[:])
                    nc.sync.dma_start(
                        rec_all[:, isl * 4 : (isl + 1) * 4],
                        dr.rearrange("(t p) -> p t", p=P),
                    )
                # ---- deferred out-proj over all 32 l-tiles
                if "tail" not in ablate and "proj" not in ablate:
                    for t in range(LT):
                        pp = stps.tile([P, 1024], f32, name="pp", tag="st")
                        nc.tensor.matmul(
                            pp[:, :512],
                            ao_all[:, t * P : (t + 1) * P],
                            wo_sb[:],
                            start=True,
                            stop=True,
                        )
                        yt = yp.tile([P, C], f32, name="yt", tag="y")
                        nc.vector.tensor_scalar_mul(
                            yt[:], pp[:, :512], rec_all[:, t : t + 1]
                        )
                        if "ydma" not in ablate:
                            nc.sync.dma_start(
                                y_d[t * P : (t + 1) * P, :], yt[:]
                            )

    nc.compile()
    return nc


_nc_cache = {}


def _get_nc(**kw):
    key = tuple(sorted(kw.items()))
    if key not in _nc_cache:
        _nc_cache[key] = build_nc(**kw)
    return _nc_cache[key]


def make_in_maps(x, w_qkv, w_out):
    """Host-side sharding: per-head weight slices, shared transposed input."""
    x = np.asarray(x, dtype=np.float32)
    w_qkv = np.asarray(w_qkv, dtype=np.float32)
    w_out = np.asarray(w_out, dtype=np.float32)
    scale = float(D) ** -0.5
    xt = np.ascontiguousarray(x[0].T).astype(_BF16)  # [C, L]
    in_maps = []
    for h in range(N_CORES):
        sl = slice(h * D, (h + 1) * D)
        wq = (w_qkv[0 * C :][sl, :] * scale).T  # [C, D]
        wk = w_qkv[1 * C :][sl, :].T
        wqk = np.ascontiguousarray(np.concatenate([wq, wk], axis=1)).astype(_BF16)
        wv = np.ascontiguousarray(w_qkv[2 * C :][sl, :].T).astype(_BF16)
        wo = np.ascontiguousarray(w_out[:, sl].T).astype(_BF16)
        in_maps.append({"xt": xt, "wqk": wqk, "wv": wv, "wo": wo})
    return in_maps


def kernel(x, w_qkv, w_out, b_out):
    from concourse.bass_utils import run_bass_kernel_spmd

    nc = _get_nc()
    in_maps = make_in_maps(x, w_qkv, w_out)
    res = run_bass_kernel_spmd(nc, in_maps, list(range(N_CORES)))
    y = res.results[0]["y"].copy()
    for i in range(1, N_CORES):
        y += res.results[i]["y"]
    y += np.asarray(b_out, dtype=np.float32)
    return y[None]

